# revision 5
# baseline (speedup 1.0000x reference)
"""Trainium2 Bass kernel for KBLAM Gemma3n attention (B=2, S=1024, H=2048,
NH=16, NKV=4, HD=128, KB=1024), sharded over 8 NeuronCores as
(batch x kv-head-group): core = 4*b + g handles batch b and kv head g
(which serves q-heads 4g..4g+3).  Each core computes a partial s-major
output y_part (S, H) = attn_out @ Wo[:, 512g:512g+512].T ; the host sums
the 4 partials per batch.

Device-side layout choices (all f32):
 - projections produce d-major tensors  qT/qnT (128*4heads? -> [128, 4096]
   head i at cols 1024i), kT [128,1024], via out = lhsT.T @ rhs with
   lhsT = W^T tile [h,d], rhs = x^T tile [h,s].
 - RoPE uses a permutation matmul (P @ qT) plus 3 DVE ops per 512-chunk.
 - scores are computed transposed: scoresT[key, q] = kT_tile.T @ qT_chunk, so
   softmax-exp output attnT feeds attn@v directly as the moving operand with
   v in natural key-major layout (no transposes in the attention inner loop).
 - softmax denominators via ones-column matmuls accumulated alongside attn@v;
   normalization via reciprocal + ones-row broadcast matmul + DVE multiply.
 - attention_mask handled generally: each 128-key x 512-query self block is
   classified on host as fully-masked (skipped), zero (no mask add), or mixed
   (additive mask tile DMA'd and added before exp).
"""
import math
from contextlib import ExitStack

import numpy as np

B, S, H = 2, 1024, 2048
NH, NKV, HD = 16, 4, 128
KB = 1024
THETA = 10000.0
SCALE = 1.0 / math.sqrt(HD)
DT = None  # set to mybir.dt.float32 lazily


def _build_program(self_tiles, mixed_idx, n_mask):
    """Build the single-core Bass/Tile program.

    self_tiles: {c: [t, ...]} self-attention key tiles to compute per q-chunk
    mixed_idx: {(t, c): k} index into the packed mask tensor for mixed blocks
    n_mask: number of packed [128, 512] mask tiles (0 if none)
    """
    import concourse.bass as bass
    import concourse.tile as tile
    from concourse import bacc, mybir

    f32 = mybir.dt.float32
    nc = bacc.Bacc("TRN2", target_bir_lowering=False, debug=False,
                   enable_asserts=False, num_devices=8)

    xT = nc.dram_tensor("xT", [H, S], f32, kind="ExternalInput")
    wq = nc.dram_tensor("wq", [H, 512], f32, kind="ExternalInput")
    wqn = nc.dram_tensor("wqn", [H, 512], f32, kind="ExternalInput")
    wk = nc.dram_tensor("wk", [H, 128], f32, kind="ExternalInput")
    wv = nc.dram_tensor("wv", [H, 128], f32, kind="ExternalInput")
    wo = nc.dram_tensor("wo", [512, H], f32, kind="ExternalInput")
    kbkT = nc.dram_tensor("kbkT", [128, KB], f32, kind="ExternalInput")
    kbv = nc.dram_tensor("kbv", [KB, 128], f32, kind="ExternalInput")
    cosT = nc.dram_tensor("cosT", [128, S], f32, kind="ExternalInput")
    sinT = nc.dram_tensor("sinT", [128, S], f32, kind="ExternalInput")
    ropePT = nc.dram_tensor("ropePT", [128, 128], f32, kind="ExternalInput")
    ones = nc.dram_tensor("ones", [128, 128], f32, kind="ExternalInput")
    ident = nc.dram_tensor("ident", [128, 128], f32, kind="ExternalInput")
    if n_mask:
        masks = nc.dram_tensor("masks", [128, 512 * n_mask], f32,
                               kind="ExternalInput")
    y = nc.dram_tensor("y", [S, H], f32, kind="ExternalOutput")

    with tile.TileContext(nc) as tc, ExitStack() as ctx:
        consts = ctx.enter_context(tc.tile_pool(name="consts", bufs=1))
        cos_sb = consts.tile([128, S], f32, tag="cos")
        nc.sync.dma_start(cos_sb[:], cosT[:])
        sin_sb = consts.tile([128, S], f32, tag="sin")
        nc.sync.dma_start(sin_sb[:], sinT[:])
        rp_sb = consts.tile([128, 128], f32, tag="rp")
        nc.sync.dma_start(rp_sb[:], ropePT[:])
        ones_sb = consts.tile([128, 128], f32, tag="ones")
        nc.sync.dma_start(ones_sb[:], ones[:])
        id_sb = consts.tile([128, 128], f32, tag="id")
        nc.sync.dma_start(id_sb[:], ident[:])
        if n_mask:
            mask_sb = consts.tile([128, 512 * n_mask], f32, tag="mask")
            nc.sync.dma_start(mask_sb[:], masks[:])

        kbp = ctx.enter_context(tc.tile_pool(name="kb", bufs=1))
        kbk_sb = kbp.tile([128, KB], f32, tag="kbk")
        nc.sync.dma_start(kbk_sb[:], kbkT[:])
        kbv_sb = kbp.tile([128, KB], f32, tag="kbv")
        for t in range(8):
            nc.sync.dma_start(kbv_sb[:, 128 * t:128 * t + 128],
                              kbv[128 * t:128 * t + 128, :])

        po = ctx.enter_context(tc.tile_pool(name="projout", bufs=1))
        qTr = po.tile([128, 4096], f32, tag="qTr")
        qnT = po.tile([128, 4096], f32, tag="qnT")
        kTr = po.tile([128, 1024], f32, tag="kTr")
        vkm = po.tile([128, 1024], f32, tag="vkm")

        # ---------------- phase 1: projections + rope + v transpose ------
        with tc.tile_pool(name="xw", bufs=1) as xw, \
             tc.tile_pool(name="wt", bufs=24) as wpool, \
             tc.tile_pool(name="ptmp", bufs=3) as ptmp, \
             tc.tile_pool(name="psp", bufs=2, space="PSUM") as psp, \
             tc.tile_pool(name="psr", bufs=2, space="PSUM") as psr:
            xt = xw.tile([128, 16384], f32, tag="xt")
            for h in range(16):
                nc.sync.dma_start(xt[:, 1024 * h:1024 * h + 1024],
                                  xT[128 * h:128 * h + 128, :])
            vt_tmp = xw.tile([128, 1024], f32, tag="vt")

            def rope_chunk(ps, half, dst):
                tmp = ptmp.tile([128, 512], f32, tag="tmp")
                nc.any.tensor_copy(tmp[:], ps[:])
                pp = psr.tile([128, 512], f32, tag="pp")
                nc.tensor.matmul(pp[:], rp_sb[:], tmp[:], start=True, stop=True)
                cs = cos_sb[:, 512 * half:512 * half + 512]
                sn = sin_sb[:, 512 * half:512 * half + 512]
                nc.vector.tensor_mul(dst, tmp[:], cs)
                tmp2 = ptmp.tile([128, 512], f32, tag="tmp2")
                nc.vector.tensor_mul(tmp2[:], pp[:], sn)
                nc.vector.tensor_add(dst, dst, tmp2[:])

            projs = [(wq, 4, 'q'), (wqn, 4, 'qn'), (wk, 1, 'k'), (wv, 1, 'v')]
            for w_dram, ndt, kind in projs:
                for dt_i in range(ndt):
                    pss = [psp.tile([128, 512], f32, tag="pp0", name="pp0"),
                           psp.tile([128, 512], f32, tag="pp1", name="pp1")]
                    for h in range(16):
                        wt_t = wpool.tile([128, 128], f32, tag="w")
                        nc.sync.dma_start(
                            wt_t[:],
                            w_dram[128 * h:128 * h + 128,
                                   128 * dt_i:128 * dt_i + 128])
                        for half in range(2):
                            nc.tensor.matmul(
                                pss[half][:], wt_t[:],
                                xt[:, 1024 * h + 512 * half:
                                   1024 * h + 512 * half + 512],
                                start=(h == 0), stop=(h == 15))
                    for half in range(2):
                        if kind == 'q':
                            dst = qTr[:, 1024 * dt_i + 512 * half:
                                      1024 * dt_i + 512 * half + 512]
                            rope_chunk(pss[half], half, dst)
                        elif kind == 'k':
                            dst = kTr[:, 512 * half:512 * half + 512]
                            rope_chunk(pss[half], half, dst)
                        elif kind == 'qn':
                            nc.any.tensor_copy(
                                qnT[:, 1024 * dt_i + 512 * half:
                                    1024 * dt_i + 512 * half + 512],
                                pss[half][:])
                        else:  # v
                            nc.any.tensor_copy(
                                vt_tmp[:, 512 * half:512 * half + 512],
                                pss[half][:])
            # transpose v to key-major
            for t in range(8):
                pst = psr.tile([128, 128], f32, tag="ptr")
                nc.tensor.transpose(pst[:], vt_tmp[:, 128 * t:128 * t + 128],
                                    id_sb[:])
                nc.any.tensor_copy(vkm[:, 128 * t:128 * t + 128], pst[:])

        # ---------------- phase 2: attention ------------------------------
        onp = ctx.enter_context(tc.tile_pool(name="onp", bufs=1))
        outn = onp.tile([128, 4096], f32, tag="outn")
        wo_sb = onp.tile([128, 8192], f32, tag="wo")
        for i in range(4):
            nc.sync.dma_start(wo_sb[:, 2048 * i:2048 * i + 2048],
                              wo[128 * i:128 * i + 128, :])

        with tc.tile_pool(name="at", bufs=6) as atp, \
             tc.tile_pool(name="nrm", bufs=4) as nrm, \
             tc.tile_pool(name="pssc", bufs=3, space="PSUM") as pssc, \
             tc.tile_pool(name="psout", bufs=2, space="PSUM") as psout, \
             tc.tile_pool(name="psden", bufs=2, space="PSUM") as psden, \
             tc.tile_pool(name="psbc", bufs=1, space="PSUM") as psbc:
            for c in range(2):
                for i in range(4):
                    qcol = 1024 * i + 512 * c
                    steps = [('kb', t) for t in range(8)] + \
                            [('sf', t) for t in self_tiles[c]]
                    nst = len(steps)
                    ops_ = psout.tile([128, 512], f32, tag="out")
                    pd = psden.tile([1, 512], f32, tag="den")
                    for j, (src, t) in enumerate(steps):
                        ps_s = pssc.tile([128, 512], f32, tag="sc")
                        if src == 'kb':
                            lhsT = kbk_sb[:, 128 * t:128 * t + 128]
                            rhs = qnT[:, qcol:qcol + 512]
                            vt_l = kbv_sb[:, 128 * t:128 * t + 128]
                        else:
                            lhsT = kTr[:, 128 * t:128 * t + 128]
                            rhs = qTr[:, qcol:qcol + 512]
                            vt_l = vkm[:, 128 * t:128 * t + 128]
                        nc.tensor.matmul(ps_s[:], lhsT, rhs,
                                         start=True, stop=True)
                        if src == 'sf' and (t, c) in mixed_idx:
                            k = mixed_idx[(t, c)]
                            nc.vector.tensor_add(
                                ps_s[:], ps_s[:],
                                mask_sb[:, 512 * k:512 * k + 512])
                        at_t = atp.tile([128, 512], f32, tag="at")
                        nc.scalar.activation(
                            at_t[:], ps_s[:],
                            mybir.ActivationFunctionType.Exp, scale=SCALE)
                        nc.tensor.matmul(ops_[:], vt_l, at_t[:],
                                         start=(j == 0), stop=(j == nst - 1))
                        nc.tensor.matmul(pd[:], ones_sb[:, 0:1], at_t[:],
                                         start=(j == 0), stop=(j == nst - 1))
                    den = nrm.tile([1, 512], f32, tag="den_sb")
                    nc.any.tensor_copy(den[:], pd[:])
                    rec = nrm.tile([1, 512], f32, tag="rec")
                    nc.vector.reciprocal(rec[:], den[:])
                    bc = psbc.tile([128, 512], f32, tag="bc")
                    nc.tensor.matmul(bc[:], ones_sb[0:1, :], rec[:],
                                     start=True, stop=True)
                    bc_sb = nrm.tile([128, 512], f32, tag="bc_sb")
                    nc.any.tensor_copy(bc_sb[:], bc[:])
                    nc.vector.tensor_mul(outn[:, qcol:qcol + 512],
                                         ops_[:], bc_sb[:])

        # ---------------- phase 3: Wo ------------------------------------
        with tc.tile_pool(name="psy", bufs=3, space="PSUM") as psy, \
             tc.tile_pool(name="ysb", bufs=4) as ysbp:
            for st in range(8):
                c, off = st // 4, 128 * (st % 4)
                for n in range(4):
                    py = psy.tile([128, 512], f32, tag="y")
                    for i in range(4):
                        lcol = 1024 * i + 512 * c + off
                        nc.tensor.matmul(
                            py[:], outn[:, lcol:lcol + 128],
                            wo_sb[:, 2048 * i + 512 * n:2048 * i + 512 * n + 512],
                            start=(i == 0), stop=(i == 3))
                    ysb = ysbp.tile([128, 512], f32, tag="ysb")
                    nc.any.tensor_copy(ysb[:], py[:])
                    nc.sync.dma_start(y[128 * st:128 * st + 128,
                                        512 * n:512 * n + 512], ysb[:])

    nc.compile()
    return nc


def kernel(hidden_states, attention_mask, position_ids, kb_keys, kb_values,
           Wq, Wq_new, Wk, Wv, Wo):
    from concourse.bass_utils import run_bass_kernel_spmd

    hidden_states = np.asarray(hidden_states, dtype=np.float32)
    attention_mask = np.asarray(attention_mask, dtype=np.float32)
    position_ids = np.asarray(position_ids)
    kb_keys = np.asarray(kb_keys, dtype=np.float32)
    kb_values = np.asarray(kb_values, dtype=np.float32)
    Wq = np.asarray(Wq, dtype=np.float32)
    Wq_new = np.asarray(Wq_new, dtype=np.float32)
    Wk = np.asarray(Wk, dtype=np.float32)
    Wv = np.asarray(Wv, dtype=np.float32)
    Wo = np.asarray(Wo, dtype=np.float32)

    # ---- host: classify self-attention mask blocks ----
    mask = attention_mask[:, 0]  # (B, S, S) [q, key]
    self_tiles = {}
    mixed = []
    for c in range(2):
        tiles = []
        for t in range(8):
            blk = mask[:, 512 * c:512 * c + 512, 128 * t:128 * t + 128]
            if np.all(blk <= -1e8):
                continue
            tiles.append(t)
            if np.any(blk < 0):
                mixed.append((t, c))
        self_tiles[c] = tiles
    mixed_idx = {tc_: k for k, tc_ in enumerate(mixed)}
    n_mask = len(mixed)

    nc = _build_program(self_tiles, mixed_idx, n_mask)

    # ---- host: shared constant prep ----
    inv_freq = 1.0 / (THETA ** (np.arange(0, HD, 2, dtype=np.float32) / HD))
    P = np.zeros((HD, HD), np.float32)
    for d in range(64):
        P[d, d + 64] = -1.0
        P[d + 64, d] = 1.0
    ropePT = np.ascontiguousarray(P.T)
    ones = np.ones((128, 128), np.float32)
    ident = np.eye(128, dtype=np.float32)

    cosTs, sinTs, maskTs = [], [], []
    for b in range(B):
        freqs = position_ids[b].astype(np.float32)[:, None] * inv_freq[None, :]
        emb = np.concatenate([freqs, freqs], axis=1)  # (S, 128)
        cosTs.append(np.ascontiguousarray(np.cos(emb).T.astype(np.float32)))
        sinTs.append(np.ascontiguousarray(np.sin(emb).T.astype(np.float32)))
        if n_mask:
            mt = np.empty((128, 512 * n_mask), np.float32)
            for (t, c), k in mixed_idx.items():
                mt[:, 512 * k:512 * k + 512] = \
                    mask[b, 512 * c:512 * c + 512, 128 * t:128 * t + 128].T
            maskTs.append(mt)

    in_maps = []
    for cid in range(8):
        b, g = cid // 4, cid % 4
        m = dict(
            xT=np.ascontiguousarray(hidden_states[b].T),
            wq=np.ascontiguousarray(Wq[512 * g:512 * g + 512, :].T),
            wqn=np.ascontiguousarray(Wq_new[512 * g:512 * g + 512, :].T),
            wk=np.ascontiguousarray(Wk[128 * g:128 * g + 128, :].T),
            wv=np.ascontiguousarray(Wv[128 * g:128 * g + 128, :].T),
            wo=np.ascontiguousarray(Wo[:, 512 * g:512 * g + 512].T),
            kbkT=np.ascontiguousarray(kb_keys[b, :, 128 * g:128 * g + 128].T),
            kbv=np.ascontiguousarray(kb_values[b, :, 128 * g:128 * g + 128]),
            cosT=cosTs[b], sinT=sinTs[b],
            ropePT=ropePT, ones=ones, ident=ident,
        )
        if n_mask:
            m['masks'] = maskTs[b]
        in_maps.append(m)

    res = run_bass_kernel_spmd(nc, in_maps, core_ids=list(range(8)))

    out = np.zeros((B, S, H), np.float32)
    for cid in range(8):
        b = cid // 4
        out[b] += res.results[cid]["y"]
    return out


# revision 7
# speedup vs baseline: 2.4410x; 2.4410x over previous
"""Trainium2 Bass kernel for KBLAM Gemma3n attention (B=2, S=1024, H=2048,
NH=16, NKV=4, HD=128, KB=1024), sharded over 8 NeuronCores as
(batch x kv-head-group): core = 4*b + g handles batch b and kv head g
(which serves q-heads 4g..4g+3).  Each core computes a partial s-major
output y_part (S, H) = attn_out @ Wo[:, 512g:512g+512].T ; the host sums
the 4 partials per batch.

Device-side layout choices (all f32):
 - projections produce d-major tensors  qT/qnT (128*4heads? -> [128, 4096]
   head i at cols 1024i), kT [128,1024], via out = lhsT.T @ rhs with
   lhsT = W^T tile [h,d], rhs = x^T tile [h,s].
 - RoPE uses a permutation matmul (P @ qT) plus 3 DVE ops per 512-chunk.
 - scores are computed transposed: scoresT[key, q] = kT_tile.T @ qT_chunk, so
   softmax-exp output attnT feeds attn@v directly as the moving operand with
   v in natural key-major layout (no transposes in the attention inner loop).
 - softmax denominators via ones-column matmuls accumulated alongside attn@v;
   normalization via reciprocal + ones-row broadcast matmul + DVE multiply.
 - attention_mask handled generally: each 128-key x 512-query self block is
   classified on host as fully-masked (skipped), zero (no mask add), or mixed
   (additive mask tile DMA'd and added before exp).
"""
import math
from contextlib import ExitStack

import numpy as np

B, S, H = 2, 1024, 2048
NH, NKV, HD = 16, 4, 128
KB = 1024
THETA = 10000.0
SCALE = 1.0 / math.sqrt(HD)
DT = None  # set to mybir.dt.float32 lazily


def _build_program(self_tiles, mixed_idx, n_mask):
    """Build the single-core Bass/Tile program.

    self_tiles: {c: [t, ...]} self-attention key tiles to compute per q-chunk
    mixed_idx: {(t, c): k} index into the packed mask tensor for mixed blocks
    n_mask: number of packed [128, 512] mask tiles (0 if none)
    """
    import concourse.bass as bass
    import concourse.tile as tile
    from concourse import bacc, mybir

    f32 = mybir.dt.float32
    f32r = mybir.dt.float32r
    nc = bacc.Bacc("TRN2", target_bir_lowering=False, debug=False,
                   enable_asserts=False, num_devices=8)

    xT = nc.dram_tensor("xT", [H, S], f32r, kind="ExternalInput")
    wq = nc.dram_tensor("wq", [H, 512], f32r, kind="ExternalInput")
    wqn = nc.dram_tensor("wqn", [H, 512], f32r, kind="ExternalInput")
    wk = nc.dram_tensor("wk", [H, 128], f32r, kind="ExternalInput")
    wv = nc.dram_tensor("wv", [H, 128], f32r, kind="ExternalInput")
    wo = nc.dram_tensor("wo", [512, H], f32r, kind="ExternalInput")
    kbkT = nc.dram_tensor("kbkT", [128, KB], f32r, kind="ExternalInput")
    kbv = nc.dram_tensor("kbv", [KB, 128], f32r, kind="ExternalInput")
    cosT = nc.dram_tensor("cosT", [128, S], f32r, kind="ExternalInput")
    sinT = nc.dram_tensor("sinT", [128, S], f32r, kind="ExternalInput")
    ropePT = nc.dram_tensor("ropePT", [128, 128], f32r, kind="ExternalInput")
    ones = nc.dram_tensor("ones", [128, 128], f32r, kind="ExternalInput")
    ident = nc.dram_tensor("ident", [128, 128], f32r, kind="ExternalInput")
    if n_mask:
        masks = nc.dram_tensor("masks", [128, 512 * n_mask], f32,
                               kind="ExternalInput")
    y = nc.dram_tensor("y", [S, H], f32, kind="ExternalOutput")

    with tile.TileContext(nc) as tc, ExitStack() as ctx:
        consts = ctx.enter_context(tc.tile_pool(name="consts", bufs=1))
        cos_sb = consts.tile([128, S], f32r, tag="cos")
        nc.sync.dma_start(cos_sb[:], cosT[:])
        sin_sb = consts.tile([128, S], f32r, tag="sin")
        nc.sync.dma_start(sin_sb[:], sinT[:])
        rp_sb = consts.tile([128, 128], f32r, tag="rp")
        nc.sync.dma_start(rp_sb[:], ropePT[:])
        ones_sb = consts.tile([128, 128], f32r, tag="ones")
        nc.sync.dma_start(ones_sb[:], ones[:])
        id_sb = consts.tile([128, 128], f32r, tag="id")
        nc.sync.dma_start(id_sb[:], ident[:])
        if n_mask:
            mask_sb = consts.tile([128, 512 * n_mask], f32, tag="mask")
            nc.sync.dma_start(mask_sb[:], masks[:])

        kbp = ctx.enter_context(tc.tile_pool(name="kb", bufs=1))
        kbk_sb = kbp.tile([128, KB], f32r, tag="kbk")
        nc.sync.dma_start(kbk_sb[:], kbkT[:])
        kbv_sb = kbp.tile([128, KB], f32r, tag="kbv")
        for t in range(8):
            nc.sync.dma_start(kbv_sb[:, 128 * t:128 * t + 128],
                              kbv[128 * t:128 * t + 128, :])

        po = ctx.enter_context(tc.tile_pool(name="projout", bufs=1))
        qTr = po.tile([128, 4096], f32r, tag="qTr")
        qnT = po.tile([128, 4096], f32r, tag="qnT")
        kTr = po.tile([128, 1024], f32r, tag="kTr")
        vkm = po.tile([128, 1024], f32r, tag="vkm")

        # ---------------- phase 1: projections + rope + v transpose ------
        with tc.tile_pool(name="xw", bufs=1) as xw, \
             tc.tile_pool(name="wt", bufs=24) as wpool, \
             tc.tile_pool(name="ptmp", bufs=3) as ptmp, \
             tc.tile_pool(name="psp", bufs=2, space="PSUM") as psp, \
             tc.tile_pool(name="psr", bufs=2, space="PSUM") as psr:
            xt = xw.tile([128, 16384], f32r, tag="xt")
            for h in range(16):
                nc.sync.dma_start(xt[:, 1024 * h:1024 * h + 1024],
                                  xT[128 * h:128 * h + 128, :])
            vt_tmp = xw.tile([128, 1024], f32r, tag="vt")

            def rope_chunk(ps, half, dst):
                tmp = ptmp.tile([128, 512], f32r, tag="tmp")
                nc.any.tensor_copy(tmp[:], ps[:])
                pp = psr.tile([128, 512], f32, tag="pp")
                nc.tensor.matmul(pp[:], rp_sb[:], tmp[:], start=True, stop=True)
                cs = cos_sb[:, 512 * half:512 * half + 512]
                sn = sin_sb[:, 512 * half:512 * half + 512]
                nc.vector.tensor_mul(dst, tmp[:], cs)
                tmp2 = ptmp.tile([128, 512], f32r, tag="tmp2")
                nc.vector.tensor_mul(tmp2[:], pp[:], sn)
                nc.vector.tensor_add(dst, dst, tmp2[:])

            projs = [(wq, 4, 'q'), (wqn, 4, 'qn'), (wk, 1, 'k'), (wv, 1, 'v')]
            for w_dram, ndt, kind in projs:
                for dt_i in range(ndt):
                    pss = [psp.tile([128, 512], f32, tag="pp0", name="pp0"),
                           psp.tile([128, 512], f32, tag="pp1", name="pp1")]
                    for h in range(16):
                        wt_t = wpool.tile([128, 128], f32r, tag="w")
                        nc.sync.dma_start(
                            wt_t[:],
                            w_dram[128 * h:128 * h + 128,
                                   128 * dt_i:128 * dt_i + 128])
                        for half in range(2):
                            nc.tensor.matmul(
                                pss[half][:], wt_t[:],
                                xt[:, 1024 * h + 512 * half:
                                   1024 * h + 512 * half + 512],
                                start=(h == 0), stop=(h == 15))
                    for half in range(2):
                        if kind == 'q':
                            dst = qTr[:, 1024 * dt_i + 512 * half:
                                      1024 * dt_i + 512 * half + 512]
                            rope_chunk(pss[half], half, dst)
                        elif kind == 'k':
                            dst = kTr[:, 512 * half:512 * half + 512]
                            rope_chunk(pss[half], half, dst)
                        elif kind == 'qn':
                            nc.any.tensor_copy(
                                qnT[:, 1024 * dt_i + 512 * half:
                                    1024 * dt_i + 512 * half + 512],
                                pss[half][:])
                        else:  # v
                            nc.any.tensor_copy(
                                vt_tmp[:, 512 * half:512 * half + 512],
                                pss[half][:])
            # transpose v to key-major
            for t in range(8):
                pst = psr.tile([128, 128], f32r, tag="ptr")
                nc.tensor.transpose(pst[:], vt_tmp[:, 128 * t:128 * t + 128],
                                    id_sb[:])
                nc.any.tensor_copy(vkm[:, 128 * t:128 * t + 128], pst[:])

        # ---------------- phase 2: attention ------------------------------
        onp = ctx.enter_context(tc.tile_pool(name="onp", bufs=1))
        outn = onp.tile([128, 4096], f32r, tag="outn")
        wo_sb = onp.tile([128, 8192], f32r, tag="wo")
        for i in range(4):
            nc.sync.dma_start(wo_sb[:, 2048 * i:2048 * i + 2048],
                              wo[128 * i:128 * i + 128, :])

        with tc.tile_pool(name="at", bufs=6) as atp, \
             tc.tile_pool(name="nrm", bufs=4) as nrm, \
             tc.tile_pool(name="pssc", bufs=3, space="PSUM") as pssc, \
             tc.tile_pool(name="psout", bufs=2, space="PSUM") as psout, \
             tc.tile_pool(name="psden", bufs=2, space="PSUM") as psden, \
             tc.tile_pool(name="psbc", bufs=1, space="PSUM") as psbc:
            for c in range(2):
                for i in range(4):
                    qcol = 1024 * i + 512 * c
                    steps = [('kb', t) for t in range(8)] + \
                            [('sf', t) for t in self_tiles[c]]
                    nst = len(steps)
                    ops_ = psout.tile([128, 512], f32, tag="out")
                    pd = psden.tile([1, 512], f32, tag="den")
                    for j, (src, t) in enumerate(steps):
                        ps_s = pssc.tile([128, 512], f32, tag="sc")
                        if src == 'kb':
                            lhsT = kbk_sb[:, 128 * t:128 * t + 128]
                            rhs = qnT[:, qcol:qcol + 512]
                            vt_l = kbv_sb[:, 128 * t:128 * t + 128]
                        else:
                            lhsT = kTr[:, 128 * t:128 * t + 128]
                            rhs = qTr[:, qcol:qcol + 512]
                            vt_l = vkm[:, 128 * t:128 * t + 128]
                        nc.tensor.matmul(ps_s[:], lhsT, rhs,
                                         start=True, stop=True)
                        if src == 'sf' and (t, c) in mixed_idx:
                            k = mixed_idx[(t, c)]
                            nc.vector.tensor_add(
                                ps_s[:], ps_s[:],
                                mask_sb[:, 512 * k:512 * k + 512])
                        at_t = atp.tile([128, 512], f32r, tag="at")
                        nc.scalar.activation(
                            at_t[:], ps_s[:],
                            mybir.ActivationFunctionType.Exp, scale=SCALE)
                        nc.tensor.matmul(ops_[:], vt_l, at_t[:],
                                         start=(j == 0), stop=(j == nst - 1))
                        nc.tensor.matmul(pd[:], ones_sb[:, 0:1], at_t[:],
                                         start=(j == 0), stop=(j == nst - 1))
                    den = nrm.tile([1, 512], f32, tag="den_sb")
                    nc.any.tensor_copy(den[:], pd[:])
                    rec = nrm.tile([1, 512], f32r, tag="rec")
                    with nc.allow_low_precision(reason="f32r is 4-byte"):
                        nc.vector.reciprocal(rec[:], den[:])
                    bc = psbc.tile([128, 512], f32, tag="bc")
                    nc.tensor.matmul(bc[:], ones_sb[0:1, :], rec[:],
                                     start=True, stop=True)
                    bc_sb = nrm.tile([128, 512], f32r, tag="bc_sb")
                    nc.any.tensor_copy(bc_sb[:], bc[:])
                    nc.vector.tensor_mul(outn[:, qcol:qcol + 512],
                                         ops_[:], bc_sb[:])

        # ---------------- phase 3: Wo ------------------------------------
        with tc.tile_pool(name="psy", bufs=3, space="PSUM") as psy, \
             tc.tile_pool(name="ysb", bufs=4) as ysbp:
            for st in range(8):
                c, off = st // 4, 128 * (st % 4)
                for n in range(4):
                    py = psy.tile([128, 512], f32, tag="y")
                    for i in range(4):
                        lcol = 1024 * i + 512 * c + off
                        nc.tensor.matmul(
                            py[:], outn[:, lcol:lcol + 128],
                            wo_sb[:, 2048 * i + 512 * n:2048 * i + 512 * n + 512],
                            start=(i == 0), stop=(i == 3))
                    ysb = ysbp.tile([128, 512], f32, tag="ysb")
                    nc.any.tensor_copy(ysb[:], py[:])
                    nc.sync.dma_start(y[128 * st:128 * st + 128,
                                        512 * n:512 * n + 512], ysb[:])

    nc.compile()
    return nc


def kernel(hidden_states, attention_mask, position_ids, kb_keys, kb_values,
           Wq, Wq_new, Wk, Wv, Wo):
    from concourse.bass_utils import run_bass_kernel_spmd

    hidden_states = np.asarray(hidden_states, dtype=np.float32)
    attention_mask = np.asarray(attention_mask, dtype=np.float32)
    position_ids = np.asarray(position_ids)
    kb_keys = np.asarray(kb_keys, dtype=np.float32)
    kb_values = np.asarray(kb_values, dtype=np.float32)
    Wq = np.asarray(Wq, dtype=np.float32)
    Wq_new = np.asarray(Wq_new, dtype=np.float32)
    Wk = np.asarray(Wk, dtype=np.float32)
    Wv = np.asarray(Wv, dtype=np.float32)
    Wo = np.asarray(Wo, dtype=np.float32)

    # ---- host: classify self-attention mask blocks ----
    mask = attention_mask[:, 0]  # (B, S, S) [q, key]
    self_tiles = {}
    mixed = []
    for c in range(2):
        tiles = []
        for t in range(8):
            blk = mask[:, 512 * c:512 * c + 512, 128 * t:128 * t + 128]
            if np.all(blk <= -1e8):
                continue
            tiles.append(t)
            if np.any(blk < 0):
                mixed.append((t, c))
        self_tiles[c] = tiles
    mixed_idx = {tc_: k for k, tc_ in enumerate(mixed)}
    n_mask = len(mixed)

    nc = _build_program(self_tiles, mixed_idx, n_mask)

    # ---- host: shared constant prep ----
    inv_freq = 1.0 / (THETA ** (np.arange(0, HD, 2, dtype=np.float32) / HD))
    P = np.zeros((HD, HD), np.float32)
    for d in range(64):
        P[d, d + 64] = -1.0
        P[d + 64, d] = 1.0
    ropePT = np.ascontiguousarray(P.T)
    ones = np.ones((128, 128), np.float32)
    ident = np.eye(128, dtype=np.float32)

    cosTs, sinTs, maskTs = [], [], []
    for b in range(B):
        freqs = position_ids[b].astype(np.float32)[:, None] * inv_freq[None, :]
        emb = np.concatenate([freqs, freqs], axis=1)  # (S, 128)
        cosTs.append(np.ascontiguousarray(np.cos(emb).T.astype(np.float32)))
        sinTs.append(np.ascontiguousarray(np.sin(emb).T.astype(np.float32)))
        if n_mask:
            mt = np.empty((128, 512 * n_mask), np.float32)
            for (t, c), k in mixed_idx.items():
                mt[:, 512 * k:512 * k + 512] = \
                    mask[b, 512 * c:512 * c + 512, 128 * t:128 * t + 128].T
            maskTs.append(mt)

    in_maps = []
    for cid in range(8):
        b, g = cid // 4, cid % 4
        m = dict(
            xT=np.ascontiguousarray(hidden_states[b].T),
            wq=np.ascontiguousarray(Wq[512 * g:512 * g + 512, :].T),
            wqn=np.ascontiguousarray(Wq_new[512 * g:512 * g + 512, :].T),
            wk=np.ascontiguousarray(Wk[128 * g:128 * g + 128, :].T),
            wv=np.ascontiguousarray(Wv[128 * g:128 * g + 128, :].T),
            wo=np.ascontiguousarray(Wo[:, 512 * g:512 * g + 512].T),
            kbkT=np.ascontiguousarray(kb_keys[b, :, 128 * g:128 * g + 128].T),
            kbv=np.ascontiguousarray(kb_values[b, :, 128 * g:128 * g + 128]),
            cosT=cosTs[b], sinT=sinTs[b],
            ropePT=ropePT, ones=ones, ident=ident,
        )
        if n_mask:
            m['masks'] = maskTs[b]
        in_maps.append(m)

    res = run_bass_kernel_spmd(nc, in_maps, core_ids=list(range(8)))

    out = np.zeros((B, S, H), np.float32)
    for cid in range(8):
        b = cid // 4
        out[b] += res.results[cid]["y"]
    return out


# revision 9
# speedup vs baseline: 2.8194x; 1.1550x over previous
"""Trainium2 Bass kernel for KBLAM Gemma3n attention (B=2, S=1024, H=2048,
NH=16, NKV=4, HD=128, KB=1024), sharded over 8 NeuronCores as
(batch x kv-head-group): core = 4*b + g handles batch b and kv head g
(which serves q-heads 4g..4g+3).  Each core computes a partial s-major
output y_part (S, H) = attn_out @ Wo[:, 512g:512g+512].T ; the host sums
the 4 partials per batch.

Device-side design (matmul operands in float32r: 4x the fp32 PE rate at
~1.6e-4 matmul relative error):
 - projections produce d-major tensors qT/qnT [128, 4096] (head i at cols
   1024i), kT [128,1024], via out = lhsT.T @ rhs with lhsT = W^T tile [h,d],
   rhs = x^T tile [h,s].  Weights are host-packed in per-dt tile order so
   each projection chunk is one contiguous [128, 2048] DMA.
 - RoPE via a permutation matmul (P @ qT) plus 3 DVE ops per 512-chunk.
 - scores are computed transposed: scoresT[key, q] = kT_tile.T @ qT_chunk, so
   softmax-exp output attnT feeds attn@v directly as the moving operand with
   v in natural key-major layout (no transposes in the attention inner loop).
 - softmax denominators via ones-column matmuls accumulated alongside attn@v;
   normalization via fast-approx reciprocal + ones-row broadcast matmul +
   DVE multiply.
 - attention_mask handled generally: each 128-key x 512-query self block is
   classified on host as fully-masked (skipped), zero (no mask add), or mixed
   (additive mask tile DMA'd and added before exp).
"""
import math
from contextlib import ExitStack

import numpy as np

B, S, H = 2, 1024, 2048
NH, NKV, HD = 16, 4, 128
KB = 1024
THETA = 10000.0
SCALE = 1.0 / math.sqrt(HD)


def _build_program(self_tiles, mixed_idx, n_mask):
    """Build the single-core Bass/Tile program.

    self_tiles: {c: [t, ...]} self-attention key tiles to compute per q-chunk
    mixed_idx: {(t, c): k} index into the packed mask tensor for mixed blocks
    n_mask: number of packed [128, 512] mask tiles (0 if none)
    """
    import concourse.tile as tile
    from concourse import bacc, mybir

    f32 = mybir.dt.float32
    f32r = mybir.dt.float32r
    nc = bacc.Bacc("TRN2", target_bir_lowering=False, debug=False,
                   enable_asserts=False, num_devices=8)

    xT = nc.dram_tensor("xT", [H, S], f32r, kind="ExternalInput")
    # packed weights: per-dt blocks of 16 h-tiles: cols 2048*dt + 128*h
    wq = nc.dram_tensor("wq", [128, 8192], f32r, kind="ExternalInput")
    wqn = nc.dram_tensor("wqn", [128, 8192], f32r, kind="ExternalInput")
    wk = nc.dram_tensor("wk", [128, 2048], f32r, kind="ExternalInput")
    wv = nc.dram_tensor("wv", [128, 2048], f32r, kind="ExternalInput")
    # wo packed: block i at cols 2048*i = Wo_g^T[128i:128i+128, :]
    wo = nc.dram_tensor("wo", [128, 8192], f32r, kind="ExternalInput")
    kbkT = nc.dram_tensor("kbkT", [128, KB], f32r, kind="ExternalInput")
    # kbv packed key-major tiles side by side: tile t at cols 128*t
    kbv = nc.dram_tensor("kbv", [128, KB], f32r, kind="ExternalInput")
    cosT = nc.dram_tensor("cosT", [128, S], f32r, kind="ExternalInput")
    sinT = nc.dram_tensor("sinT", [128, S], f32r, kind="ExternalInput")
    ropePT = nc.dram_tensor("ropePT", [128, 128], f32r, kind="ExternalInput")
    ones = nc.dram_tensor("ones", [128, 128], f32r, kind="ExternalInput")
    ident = nc.dram_tensor("ident", [128, 128], f32r, kind="ExternalInput")
    if n_mask:
        masks = nc.dram_tensor("masks", [128, 512 * n_mask], f32,
                               kind="ExternalInput")
    y = nc.dram_tensor("y", [S, H], f32, kind="ExternalOutput")

    with tile.TileContext(nc) as tc, ExitStack() as ctx:
        po = ctx.enter_context(tc.tile_pool(name="projout", bufs=1))
        qTr = po.tile([128, 4096], f32r, tag="qTr")
        qnT = po.tile([128, 4096], f32r, tag="qnT")
        kTr = po.tile([128, 1024], f32r, tag="kTr")
        vkm = po.tile([128, 1024], f32r, tag="vkm")

        consts = ctx.enter_context(tc.tile_pool(name="consts", bufs=1))
        kbp = ctx.enter_context(tc.tile_pool(name="kb", bufs=1))

        # ---------------- phase 1: projections + rope + v transpose ------
        with tc.tile_pool(name="xw", bufs=1) as xw, \
             tc.tile_pool(name="wt", bufs=3) as wpool, \
             tc.tile_pool(name="ptmp", bufs=3) as ptmp, \
             tc.tile_pool(name="psp", bufs=2, space="PSUM") as psp, \
             tc.tile_pool(name="psr", bufs=2, space="PSUM") as psr:
            xt = xw.tile([128, 16384], f32r, tag="xt")
            for h in range(16):
                nc.sync.dma_start(xt[:, 1024 * h:1024 * h + 1024],
                                  xT[128 * h:128 * h + 128, :])
            vt_tmp = xw.tile([128, 1024], f32r, tag="vt")

            cos_sb = consts.tile([128, S], f32r, tag="cos")
            nc.sync.dma_start(cos_sb[:], cosT[:])
            sin_sb = consts.tile([128, S], f32r, tag="sin")
            nc.sync.dma_start(sin_sb[:], sinT[:])
            rp_sb = consts.tile([128, 128], f32r, tag="rp")
            nc.sync.dma_start(rp_sb[:], ropePT[:])
            id_sb = consts.tile([128, 128], f32r, tag="id")
            nc.sync.dma_start(id_sb[:], ident[:])

            def rope_chunk(ps, half, dst):
                tmp = ptmp.tile([128, 512], f32r, tag="tmp")
                nc.any.tensor_copy(tmp[:], ps[:])
                pp = psr.tile([128, 512], f32, tag="pp")
                nc.tensor.matmul(pp[:], rp_sb[:], tmp[:], start=True, stop=True)
                cs = cos_sb[:, 512 * half:512 * half + 512]
                sn = sin_sb[:, 512 * half:512 * half + 512]
                nc.vector.tensor_mul(dst, tmp[:], cs)
                tmp2 = ptmp.tile([128, 512], f32r, tag="tmp2")
                nc.vector.tensor_mul(tmp2[:], pp[:], sn)
                nc.vector.tensor_add(dst, dst, tmp2[:])

            # order: k, v first, then (q, qn) per head: lets attention for
            # head i start while head i+1 is still projecting.
            chunks = [(wk, 0, 'k'), (wv, 0, 'v')]
            for i in range(4):
                chunks.append((wq, i, 'q'))
                chunks.append((wqn, i, 'qn'))
            for w_dram, dt_i, kind in chunks:
                wblk = wpool.tile([128, 2048], f32r, tag="wblk")
                nc.sync.dma_start(wblk[:],
                                  w_dram[:, 2048 * dt_i:2048 * dt_i + 2048])
                pss = [psp.tile([128, 512], f32, tag="pp0", name="pp0"),
                       psp.tile([128, 512], f32, tag="pp1", name="pp1")]
                for h in range(16):
                    for half in range(2):
                        nc.tensor.matmul(
                            pss[half][:], wblk[:, 128 * h:128 * h + 128],
                            xt[:, 1024 * h + 512 * half:
                               1024 * h + 512 * half + 512],
                            start=(h == 0), stop=(h == 15))
                for half in range(2):
                    if kind == 'q':
                        dst = qTr[:, 1024 * dt_i + 512 * half:
                                  1024 * dt_i + 512 * half + 512]
                        rope_chunk(pss[half], half, dst)
                    elif kind == 'k':
                        dst = kTr[:, 512 * half:512 * half + 512]
                        rope_chunk(pss[half], half, dst)
                    elif kind == 'qn':
                        nc.any.tensor_copy(
                            qnT[:, 1024 * dt_i + 512 * half:
                                1024 * dt_i + 512 * half + 512],
                            pss[half][:])
                    else:  # v
                        nc.any.tensor_copy(
                            vt_tmp[:, 512 * half:512 * half + 512],
                            pss[half][:])
                if kind == 'v':
                    for t in range(8):
                        pst = psr.tile([128, 128], f32r, tag="ptr")
                        nc.tensor.transpose(
                            pst[:], vt_tmp[:, 128 * t:128 * t + 128], id_sb[:])
                        nc.any.tensor_copy(vkm[:, 128 * t:128 * t + 128],
                                           pst[:])

            # loads needed by the attention phase (emitted late so the
            # projection-critical DMAs win the early queue slots)
            ones_sb = consts.tile([128, 128], f32r, tag="ones")
            nc.sync.dma_start(ones_sb[:], ones[:])
            if n_mask:
                mask_sb = consts.tile([128, 512 * n_mask], f32, tag="mask")
                nc.sync.dma_start(mask_sb[:], masks[:])
            kbk_sb = kbp.tile([128, KB], f32r, tag="kbk")
            nc.sync.dma_start(kbk_sb[:], kbkT[:])
            kbv_sb = kbp.tile([128, KB], f32r, tag="kbv")
            nc.sync.dma_start(kbv_sb[:], kbv[:])

        # ---------------- phase 2: attention ------------------------------
        onp = ctx.enter_context(tc.tile_pool(name="onp", bufs=1))
        outn = onp.tile([128, 4096], f32r, tag="outn")
        wo_sb = onp.tile([128, 8192], f32r, tag="wo")
        nc.sync.dma_start(wo_sb[:], wo[:])

        with tc.tile_pool(name="at", bufs=6) as atp, \
             tc.tile_pool(name="nrm", bufs=3) as nrm, \
             tc.tile_pool(name="pssc", bufs=3, space="PSUM") as pssc, \
             tc.tile_pool(name="psout", bufs=3, space="PSUM") as psout, \
             tc.tile_pool(name="psden", bufs=1, space="PSUM") as psden, \
             tc.tile_pool(name="psbc", bufs=1, space="PSUM") as psbc:
            for c in range(2):
                for i in range(4):
                    qcol = 1024 * i + 512 * c
                    steps = [('kb', t) for t in range(8)] + \
                            [('sf', t) for t in self_tiles[c]]
                    nst = len(steps)
                    ops_ = psout.tile([128, 512], f32, tag="out")
                    pd = psden.tile([1, 512], f32, tag="den")
                    for j, (src, t) in enumerate(steps):
                        ps_s = pssc.tile([128, 512], f32, tag="sc")
                        if src == 'kb':
                            lhsT = kbk_sb[:, 128 * t:128 * t + 128]
                            rhs = qnT[:, qcol:qcol + 512]
                            vt_l = kbv_sb[:, 128 * t:128 * t + 128]
                        else:
                            lhsT = kTr[:, 128 * t:128 * t + 128]
                            rhs = qTr[:, qcol:qcol + 512]
                            vt_l = vkm[:, 128 * t:128 * t + 128]
                        nc.tensor.matmul(ps_s[:], lhsT, rhs,
                                         start=True, stop=True)
                        if src == 'sf' and (t, c) in mixed_idx:
                            k = mixed_idx[(t, c)]
                            nc.vector.tensor_add(
                                ps_s[:], ps_s[:],
                                mask_sb[:, 512 * k:512 * k + 512])
                        at_t = atp.tile([128, 512], f32r, tag="at")
                        nc.scalar.activation(
                            at_t[:], ps_s[:],
                            mybir.ActivationFunctionType.Exp, scale=SCALE)
                        nc.tensor.matmul(ops_[:], vt_l, at_t[:],
                                         start=(j == 0), stop=(j == nst - 1))
                        nc.tensor.matmul(pd[:], ones_sb[:, 0:1], at_t[:],
                                         start=(j == 0), stop=(j == nst - 1))
                    den = nrm.tile([1, 512], f32, tag="den_sb")
                    nc.any.tensor_copy(den[:], pd[:])
                    rec32 = nrm.tile([1, 512], f32, tag="rec32")
                    nc.vector.reciprocal_approx_fast(rec32[:], den[:])
                    rec = nrm.tile([1, 512], f32r, tag="rec")
                    nc.any.tensor_copy(rec[:], rec32[:])
                    bc = psbc.tile([128, 512], f32, tag="bc")
                    nc.tensor.matmul(bc[:], ones_sb[0:1, :], rec[:],
                                     start=True, stop=True)
                    bc_sb = nrm.tile([128, 512], f32r, tag="bc_sb")
                    nc.any.tensor_copy(bc_sb[:], bc[:])
                    nc.vector.tensor_mul(outn[:, qcol:qcol + 512],
                                         ops_[:], bc_sb[:])

        # ---------------- phase 3: Wo ------------------------------------
        with tc.tile_pool(name="psy", bufs=3, space="PSUM") as psy, \
             tc.tile_pool(name="ysb", bufs=2) as ysbp:
            for st in range(8):
                c, off = st // 4, 128 * (st % 4)
                ysb = ysbp.tile([128, 2048], f32, tag="ysb")
                for n in range(4):
                    py = psy.tile([128, 512], f32, tag="y")
                    for i in range(4):
                        lcol = 1024 * i + 512 * c + off
                        nc.tensor.matmul(
                            py[:], outn[:, lcol:lcol + 128],
                            wo_sb[:, 2048 * i + 512 * n:2048 * i + 512 * n + 512],
                            start=(i == 0), stop=(i == 3))
                    nc.any.tensor_copy(ysb[:, 512 * n:512 * n + 512], py[:])
                nc.sync.dma_start(y[128 * st:128 * st + 128, :], ysb[:])

    nc.compile()
    return nc


def kernel(hidden_states, attention_mask, position_ids, kb_keys, kb_values,
           Wq, Wq_new, Wk, Wv, Wo):
    from concourse.bass_utils import run_bass_kernel_spmd

    hidden_states = np.asarray(hidden_states, dtype=np.float32)
    attention_mask = np.asarray(attention_mask, dtype=np.float32)
    position_ids = np.asarray(position_ids)
    kb_keys = np.asarray(kb_keys, dtype=np.float32)
    kb_values = np.asarray(kb_values, dtype=np.float32)
    Wq = np.asarray(Wq, dtype=np.float32)
    Wq_new = np.asarray(Wq_new, dtype=np.float32)
    Wk = np.asarray(Wk, dtype=np.float32)
    Wv = np.asarray(Wv, dtype=np.float32)
    Wo = np.asarray(Wo, dtype=np.float32)

    # ---- host: classify self-attention mask blocks ----
    mask = attention_mask[:, 0]  # (B, S, S) [q, key]
    self_tiles = {}
    mixed = []
    for c in range(2):
        tiles = []
        for t in range(8):
            blk = mask[:, 512 * c:512 * c + 512, 128 * t:128 * t + 128]
            if np.all(blk <= -1e8):
                continue
            tiles.append(t)
            if np.any(blk < 0):
                mixed.append((t, c))
        self_tiles[c] = tiles
    mixed_idx = {tc_: k for k, tc_ in enumerate(mixed)}
    n_mask = len(mixed)

    nc = _build_program(self_tiles, mixed_idx, n_mask)

    # ---- host: shared constant prep ----
    inv_freq = 1.0 / (THETA ** (np.arange(0, HD, 2, dtype=np.float32) / HD))
    P = np.zeros((HD, HD), np.float32)
    for d in range(64):
        P[d, d + 64] = -1.0
        P[d + 64, d] = 1.0
    ropePT = np.ascontiguousarray(P.T)
    ones = np.ones((128, 128), np.float32)
    ident = np.eye(128, dtype=np.float32)

    def pack_w(wT, ndt):
        # wT (H, 128*ndt) -> (128, 2048*ndt): tile (dt) block holds 16
        # h-tiles side by side: cols 2048*dt + 128*h = wT[128h:+128, 128dt:+128]
        out = np.empty((128, 2048 * ndt), np.float32)
        for dt_i in range(ndt):
            for h in range(16):
                out[:, 2048 * dt_i + 128 * h:2048 * dt_i + 128 * h + 128] = \
                    wT[128 * h:128 * h + 128, 128 * dt_i:128 * dt_i + 128]
        return out

    cosTs, sinTs, maskTs = [], [], []
    for b in range(B):
        freqs = position_ids[b].astype(np.float32)[:, None] * inv_freq[None, :]
        emb = np.concatenate([freqs, freqs], axis=1)  # (S, 128)
        cosTs.append(np.ascontiguousarray(np.cos(emb).T.astype(np.float32)))
        sinTs.append(np.ascontiguousarray(np.sin(emb).T.astype(np.float32)))
        if n_mask:
            mt = np.empty((128, 512 * n_mask), np.float32)
            for (t, c), k in mixed_idx.items():
                mt[:, 512 * k:512 * k + 512] = \
                    mask[b, 512 * c:512 * c + 512, 128 * t:128 * t + 128].T
            maskTs.append(mt)

    in_maps = []
    for cid in range(8):
        b, g = cid // 4, cid % 4
        kbv_p = np.empty((128, KB), np.float32)
        kvb = kb_values[b, :, 128 * g:128 * g + 128]
        for t in range(8):
            kbv_p[:, 128 * t:128 * t + 128] = kvb[128 * t:128 * t + 128, :]
        wo_p = np.empty((128, 8192), np.float32)
        woT = Wo[:, 512 * g:512 * g + 512].T  # (512, 2048)
        for i in range(4):
            wo_p[:, 2048 * i:2048 * i + 2048] = woT[128 * i:128 * i + 128, :]
        m = dict(
            xT=np.ascontiguousarray(hidden_states[b].T),
            wq=pack_w(Wq[512 * g:512 * g + 512, :].T, 4),
            wqn=pack_w(Wq_new[512 * g:512 * g + 512, :].T, 4),
            wk=pack_w(Wk[128 * g:128 * g + 128, :].T, 1),
            wv=pack_w(Wv[128 * g:128 * g + 128, :].T, 1),
            wo=wo_p,
            kbkT=np.ascontiguousarray(kb_keys[b, :, 128 * g:128 * g + 128].T),
            kbv=kbv_p,
            cosT=cosTs[b], sinT=sinTs[b],
            ropePT=ropePT, ones=ones, ident=ident,
        )
        if n_mask:
            m['masks'] = maskTs[b]
        in_maps.append(m)

    res = run_bass_kernel_spmd(nc, in_maps, core_ids=list(range(8)))

    out = np.zeros((B, S, H), np.float32)
    for cid in range(8):
        b = cid // 4
        out[b] += res.results[cid]["y"]
    return out


# revision 10
# speedup vs baseline: 3.4770x; 1.2332x over previous
"""Trainium2 Bass kernel for KBLAM Gemma3n attention (B=2, S=1024, H=2048,
NH=16, NKV=4, HD=128, KB=1024), sharded over 8 NeuronCores as
(batch x kv-head-group): core = 4*b + g handles batch b and kv head g
(which serves q-heads 4g..4g+3).  Each core computes a partial s-major
output y_part (S, H) = attn_out @ Wo[:, 512g:512g+512].T ; the host sums
the 4 partials per batch.

Device-side design (matmul operands in float32r: 4x the fp32 PE rate at
~1.6e-4 matmul relative error):
 - projections produce d-major tensors qT/qnT [128, 4096] (head i at cols
   1024i), kT [128,1024], via out = lhsT.T @ rhs with lhsT = W^T tile [h,d],
   rhs = x^T tile [h,s].  Weights are host-packed in per-dt tile order so
   each projection chunk is one contiguous [128, 2048] DMA.
 - RoPE via a permutation matmul (P @ qT) plus 3 DVE ops per 512-chunk.
 - scores are computed transposed: scoresT[key, q] = kT_tile.T @ qT_chunk, so
   softmax-exp output attnT feeds attn@v directly as the moving operand with
   v in natural key-major layout (no transposes in the attention inner loop).
 - softmax denominators via ones-column matmuls accumulated alongside attn@v;
   normalization via fast-approx reciprocal + ones-row broadcast matmul +
   DVE multiply.
 - attention_mask handled generally: each 128-key x 512-query self block is
   classified on host as fully-masked (skipped), zero (no mask add), or mixed
   (additive mask tile DMA'd and added before exp).
"""
import math
from contextlib import ExitStack

import numpy as np

B, S, H = 2, 1024, 2048
NH, NKV, HD = 16, 4, 128
KB = 1024
THETA = 10000.0
SCALE = 1.0 / math.sqrt(HD)


def _build_program(self_tiles, mixed_idx, n_mask):
    """Build the single-core Bass/Tile program.

    self_tiles: {c: [t, ...]} self-attention key tiles to compute per q-chunk
    mixed_idx: {(t, c): k} index into the packed mask tensor for mixed blocks
    n_mask: number of packed [128, 512] mask tiles (0 if none)
    """
    import concourse.tile as tile
    from concourse import bacc, mybir

    f32 = mybir.dt.float32
    f32r = mybir.dt.float32r
    nc = bacc.Bacc("TRN2", target_bir_lowering=False, debug=False,
                   enable_asserts=False, num_devices=8)

    xT = nc.dram_tensor("xT", [H, S], f32r, kind="ExternalInput")
    # packed weights: per-dt blocks of 16 h-tiles: cols 2048*dt + 128*h
    wq = nc.dram_tensor("wq", [128, 8192], f32r, kind="ExternalInput")
    wqn = nc.dram_tensor("wqn", [128, 8192], f32r, kind="ExternalInput")
    wk = nc.dram_tensor("wk", [128, 2048], f32r, kind="ExternalInput")
    wv = nc.dram_tensor("wv", [128, 2048], f32r, kind="ExternalInput")
    # wo packed: block i at cols 2048*i = Wo_g^T[128i:128i+128, :]
    wo = nc.dram_tensor("wo", [128, 8192], f32r, kind="ExternalInput")
    kbkT = nc.dram_tensor("kbkT", [128, KB], f32r, kind="ExternalInput")
    # kbv packed key-major tiles side by side: tile t at cols 128*t
    kbv = nc.dram_tensor("kbv", [128, KB], f32r, kind="ExternalInput")
    cosT = nc.dram_tensor("cosT", [128, S], f32r, kind="ExternalInput")
    sinT = nc.dram_tensor("sinT", [128, S], f32r, kind="ExternalInput")
    ropePT = nc.dram_tensor("ropePT", [128, 128], f32r, kind="ExternalInput")
    ones = nc.dram_tensor("ones", [128, 128], f32r, kind="ExternalInput")
    ident = nc.dram_tensor("ident", [128, 128], f32r, kind="ExternalInput")
    if n_mask:
        masks = nc.dram_tensor("masks", [128, 512 * n_mask], f32,
                               kind="ExternalInput")
    y = nc.dram_tensor("y", [S, H], f32, kind="ExternalOutput")

    with tile.TileContext(nc) as tc, ExitStack() as ctx:
        po = ctx.enter_context(tc.tile_pool(name="projout", bufs=1))
        qTr = po.tile([128, 4096], f32r, tag="qTr")
        qnT = po.tile([128, 4096], f32r, tag="qnT")
        kTr = po.tile([128, 1024], f32r, tag="kTr")
        vkm = po.tile([128, 1024], f32r, tag="vkm")

        consts = ctx.enter_context(tc.tile_pool(name="consts", bufs=1))
        kbp = ctx.enter_context(tc.tile_pool(name="kb", bufs=1))

        # ---------------- phase 1: projections + rope + v transpose ------
        with tc.tile_pool(name="xw", bufs=1) as xw, \
             tc.tile_pool(name="wt", bufs=3) as wpool, \
             tc.tile_pool(name="ptmp", bufs=3) as ptmp, \
             tc.tile_pool(name="psp", bufs=2, space="PSUM") as psp, \
             tc.tile_pool(name="psr", bufs=2, space="PSUM") as psr:
            # weight blocks for the first two chunks (k, v) load BEFORE
            # the big xT transfer so the PE can start at ~3us.
            wblk_k = wpool.tile([128, 2048], f32r, tag="wblk", name="wblk_k")
            nc.sync.dma_start(wblk_k[:], wk[:])
            wblk_v = wpool.tile([128, 2048], f32r, tag="wblk", name="wblk_v")
            nc.sync.dma_start(wblk_v[:], wv[:])
            xt = xw.tile([128, 16384], f32r, tag="xt")
            for h in range(16):
                nc.sync.dma_start(xt[:, 1024 * h:1024 * h + 1024],
                                  xT[128 * h:128 * h + 128, :])
            vt_tmp = xw.tile([128, 1024], f32r, tag="vt")

            cos_sb = consts.tile([128, S], f32r, tag="cos")
            nc.sync.dma_start(cos_sb[:], cosT[:])
            sin_sb = consts.tile([128, S], f32r, tag="sin")
            nc.sync.dma_start(sin_sb[:], sinT[:])
            rp_sb = consts.tile([128, 128], f32r, tag="rp")
            nc.sync.dma_start(rp_sb[:], ropePT[:])
            id_sb = consts.tile([128, 128], f32r, tag="id")
            nc.sync.dma_start(id_sb[:], ident[:])

            def rope_chunk(ps, half, dst):
                tmp = ptmp.tile([128, 512], f32r, tag="tmp")
                nc.any.tensor_copy(tmp[:], ps[:])
                pp = psr.tile([128, 512], f32, tag="pp")
                nc.tensor.matmul(pp[:], rp_sb[:], tmp[:], start=True, stop=True)
                cs = cos_sb[:, 512 * half:512 * half + 512]
                sn = sin_sb[:, 512 * half:512 * half + 512]
                nc.vector.tensor_mul(dst, tmp[:], cs)
                tmp2 = ptmp.tile([128, 512], f32r, tag="tmp2")
                nc.vector.tensor_mul(tmp2[:], pp[:], sn)
                nc.vector.tensor_add(dst, dst, tmp2[:])

            # order: k, v first, then (q, qn) per head: lets attention for
            # head i start while head i+1 is still projecting.
            chunks = [(wk, 0, 'k'), (wv, 0, 'v')]
            for i in range(4):
                chunks.append((wq, i, 'q'))
                chunks.append((wqn, i, 'qn'))
            for ci, (w_dram, dt_i, kind) in enumerate(chunks):
                if ci == 0:
                    wblk = wblk_k
                elif ci == 1:
                    wblk = wblk_v
                else:
                    wblk = wpool.tile([128, 2048], f32r, tag="wblk",
                                      name="wblk")
                    nc.sync.dma_start(wblk[:],
                                      w_dram[:, 2048 * dt_i:2048 * dt_i + 2048])
                pss = [psp.tile([128, 512], f32, tag="pp0", name="pp0"),
                       psp.tile([128, 512], f32, tag="pp1", name="pp1")]
                for h in range(16):
                    for half in range(2):
                        nc.tensor.matmul(
                            pss[half][:], wblk[:, 128 * h:128 * h + 128],
                            xt[:, 1024 * h + 512 * half:
                               1024 * h + 512 * half + 512],
                            start=(h == 0), stop=(h == 15))
                for half in range(2):
                    if kind == 'q':
                        dst = qTr[:, 1024 * dt_i + 512 * half:
                                  1024 * dt_i + 512 * half + 512]
                        rope_chunk(pss[half], half, dst)
                    elif kind == 'k':
                        dst = kTr[:, 512 * half:512 * half + 512]
                        rope_chunk(pss[half], half, dst)
                    elif kind == 'qn':
                        nc.any.tensor_copy(
                            qnT[:, 1024 * dt_i + 512 * half:
                                1024 * dt_i + 512 * half + 512],
                            pss[half][:])
                    else:  # v
                        nc.any.tensor_copy(
                            vt_tmp[:, 512 * half:512 * half + 512],
                            pss[half][:])
                if kind == 'v':
                    for t in range(8):
                        pst = psr.tile([128, 128], f32r, tag="ptr")
                        nc.tensor.transpose(
                            pst[:], vt_tmp[:, 128 * t:128 * t + 128], id_sb[:])
                        nc.any.tensor_copy(vkm[:, 128 * t:128 * t + 128],
                                           pst[:])

            # loads needed by the attention phase (emitted late so the
            # projection-critical DMAs win the early queue slots)
            ones_sb = consts.tile([128, 128], f32r, tag="ones")
            nc.sync.dma_start(ones_sb[:], ones[:])
            if n_mask:
                mask_sb = consts.tile([128, 512 * n_mask], f32, tag="mask")
                nc.sync.dma_start(mask_sb[:], masks[:])
            kbk_sb = kbp.tile([128, KB], f32r, tag="kbk")
            nc.sync.dma_start(kbk_sb[:], kbkT[:])
            kbv_sb = kbp.tile([128, KB], f32r, tag="kbv")
            nc.sync.dma_start(kbv_sb[:], kbv[:])

        # ---------------- phase 2: attention ------------------------------
        onp = ctx.enter_context(tc.tile_pool(name="onp", bufs=1))
        outn = onp.tile([128, 4096], f32r, tag="outn")
        wo_sb = onp.tile([128, 8192], f32r, tag="wo")
        nc.sync.dma_start(wo_sb[:], wo[:])

        with tc.tile_pool(name="at", bufs=6) as atp, \
             tc.tile_pool(name="nrm", bufs=3) as nrm, \
             tc.tile_pool(name="pssc", bufs=3, space="PSUM") as pssc, \
             tc.tile_pool(name="psout", bufs=3, space="PSUM") as psout, \
             tc.tile_pool(name="psden", bufs=1, space="PSUM") as psden, \
             tc.tile_pool(name="psbc", bufs=1, space="PSUM") as psbc:
            for c in range(2):
                for i in range(4):
                    qcol = 1024 * i + 512 * c
                    steps = [('kb', t) for t in range(8)] + \
                            [('sf', t) for t in self_tiles[c]]
                    nst = len(steps)
                    ops_ = psout.tile([128, 512], f32, tag="out")
                    pd = psden.tile([1, 512], f32, tag="den")
                    pending = None  # (at_t, vt_l) awaiting attn@v/denominator

                    def flush(last):
                        at_p, vt_p, jj = pending
                        nc.tensor.matmul(ops_[:], vt_p, at_p[:],
                                         start=(jj == 0), stop=last)
                        nc.tensor.matmul(pd[:], ones_sb[:, 0:1], at_p[:],
                                         start=(jj == 0), stop=last)

                    for j, (src, t) in enumerate(steps):
                        ps_s = pssc.tile([128, 512], f32, tag="sc")
                        if src == 'kb':
                            lhsT = kbk_sb[:, 128 * t:128 * t + 128]
                            rhs = qnT[:, qcol:qcol + 512]
                            vt_l = kbv_sb[:, 128 * t:128 * t + 128]
                        else:
                            lhsT = kTr[:, 128 * t:128 * t + 128]
                            rhs = qTr[:, qcol:qcol + 512]
                            vt_l = vkm[:, 128 * t:128 * t + 128]
                        nc.tensor.matmul(ps_s[:], lhsT, rhs,
                                         start=True, stop=True)
                        if src == 'sf' and (t, c) in mixed_idx:
                            k = mixed_idx[(t, c)]
                            nc.vector.tensor_add(
                                ps_s[:], ps_s[:],
                                mask_sb[:, 512 * k:512 * k + 512])
                        at_t = atp.tile([128, 512], f32r, tag="at")
                        nc.scalar.activation(
                            at_t[:], ps_s[:],
                            mybir.ActivationFunctionType.Exp, scale=SCALE)
                        if pending is not None:
                            flush(False)
                        pending = (at_t, vt_l, j)
                    flush(True)
                    den = nrm.tile([1, 512], f32, tag="den_sb")
                    nc.vector.tensor_copy(den[:], pd[:])
                    rec32 = nrm.tile([1, 512], f32, tag="rec32")
                    nc.vector.reciprocal_approx_fast(rec32[:], den[:])
                    rec = nrm.tile([1, 512], f32r, tag="rec")
                    nc.vector.tensor_copy(rec[:], rec32[:])
                    bc = psbc.tile([128, 512], f32, tag="bc")
                    nc.tensor.matmul(bc[:], ones_sb[0:1, :], rec[:],
                                     start=True, stop=True)
                    bc_sb = nrm.tile([128, 512], f32r, tag="bc_sb")
                    nc.vector.tensor_copy(bc_sb[:], bc[:])
                    nc.vector.tensor_mul(outn[:, qcol:qcol + 512],
                                         ops_[:], bc_sb[:])

        # ---------------- phase 3: Wo ------------------------------------
        with tc.tile_pool(name="psy", bufs=3, space="PSUM") as psy, \
             tc.tile_pool(name="ysb", bufs=2) as ysbp:
            for st in range(8):
                c, off = st // 4, 128 * (st % 4)
                ysb = ysbp.tile([128, 2048], f32, tag="ysb")
                for n in range(4):
                    py = psy.tile([128, 512], f32, tag="y")
                    for i in range(4):
                        lcol = 1024 * i + 512 * c + off
                        nc.tensor.matmul(
                            py[:], outn[:, lcol:lcol + 128],
                            wo_sb[:, 2048 * i + 512 * n:2048 * i + 512 * n + 512],
                            start=(i == 0), stop=(i == 3))
                    nc.vector.tensor_copy(ysb[:, 512 * n:512 * n + 512], py[:])
                nc.sync.dma_start(y[128 * st:128 * st + 128, :], ysb[:])

    nc.compile()
    return nc


def kernel(hidden_states, attention_mask, position_ids, kb_keys, kb_values,
           Wq, Wq_new, Wk, Wv, Wo):
    from concourse.bass_utils import run_bass_kernel_spmd

    hidden_states = np.asarray(hidden_states, dtype=np.float32)
    attention_mask = np.asarray(attention_mask, dtype=np.float32)
    position_ids = np.asarray(position_ids)
    kb_keys = np.asarray(kb_keys, dtype=np.float32)
    kb_values = np.asarray(kb_values, dtype=np.float32)
    Wq = np.asarray(Wq, dtype=np.float32)
    Wq_new = np.asarray(Wq_new, dtype=np.float32)
    Wk = np.asarray(Wk, dtype=np.float32)
    Wv = np.asarray(Wv, dtype=np.float32)
    Wo = np.asarray(Wo, dtype=np.float32)

    # ---- host: classify self-attention mask blocks ----
    mask = attention_mask[:, 0]  # (B, S, S) [q, key]
    self_tiles = {}
    mixed = []
    for c in range(2):
        tiles = []
        for t in range(8):
            blk = mask[:, 512 * c:512 * c + 512, 128 * t:128 * t + 128]
            if np.all(blk <= -1e8):
                continue
            tiles.append(t)
            if np.any(blk < 0):
                mixed.append((t, c))
        self_tiles[c] = tiles
    mixed_idx = {tc_: k for k, tc_ in enumerate(mixed)}
    n_mask = len(mixed)

    nc = _build_program(self_tiles, mixed_idx, n_mask)

    # ---- host: shared constant prep ----
    inv_freq = 1.0 / (THETA ** (np.arange(0, HD, 2, dtype=np.float32) / HD))
    P = np.zeros((HD, HD), np.float32)
    for d in range(64):
        P[d, d + 64] = -1.0
        P[d + 64, d] = 1.0
    ropePT = np.ascontiguousarray(P.T)
    ones = np.ones((128, 128), np.float32)
    ident = np.eye(128, dtype=np.float32)

    def pack_w(wT, ndt):
        # wT (H, 128*ndt) -> (128, 2048*ndt): tile (dt) block holds 16
        # h-tiles side by side: cols 2048*dt + 128*h = wT[128h:+128, 128dt:+128]
        out = np.empty((128, 2048 * ndt), np.float32)
        for dt_i in range(ndt):
            for h in range(16):
                out[:, 2048 * dt_i + 128 * h:2048 * dt_i + 128 * h + 128] = \
                    wT[128 * h:128 * h + 128, 128 * dt_i:128 * dt_i + 128]
        return out

    cosTs, sinTs, maskTs = [], [], []
    for b in range(B):
        freqs = position_ids[b].astype(np.float32)[:, None] * inv_freq[None, :]
        emb = np.concatenate([freqs, freqs], axis=1)  # (S, 128)
        cosTs.append(np.ascontiguousarray(np.cos(emb).T.astype(np.float32)))
        sinTs.append(np.ascontiguousarray(np.sin(emb).T.astype(np.float32)))
        if n_mask:
            mt = np.empty((128, 512 * n_mask), np.float32)
            for (t, c), k in mixed_idx.items():
                mt[:, 512 * k:512 * k + 512] = \
                    mask[b, 512 * c:512 * c + 512, 128 * t:128 * t + 128].T
            maskTs.append(mt)

    in_maps = []
    for cid in range(8):
        b, g = cid // 4, cid % 4
        kbv_p = np.empty((128, KB), np.float32)
        kvb = kb_values[b, :, 128 * g:128 * g + 128]
        for t in range(8):
            kbv_p[:, 128 * t:128 * t + 128] = kvb[128 * t:128 * t + 128, :]
        wo_p = np.empty((128, 8192), np.float32)
        woT = Wo[:, 512 * g:512 * g + 512].T  # (512, 2048)
        for i in range(4):
            wo_p[:, 2048 * i:2048 * i + 2048] = woT[128 * i:128 * i + 128, :]
        m = dict(
            xT=np.ascontiguousarray(hidden_states[b].T),
            wq=pack_w(Wq[512 * g:512 * g + 512, :].T, 4),
            wqn=pack_w(Wq_new[512 * g:512 * g + 512, :].T, 4),
            wk=pack_w(Wk[128 * g:128 * g + 128, :].T, 1),
            wv=pack_w(Wv[128 * g:128 * g + 128, :].T, 1),
            wo=wo_p,
            kbkT=np.ascontiguousarray(kb_keys[b, :, 128 * g:128 * g + 128].T),
            kbv=kbv_p,
            cosT=cosTs[b], sinT=sinTs[b],
            ropePT=ropePT, ones=ones, ident=ident,
        )
        if n_mask:
            m['masks'] = maskTs[b]
        in_maps.append(m)

    res = run_bass_kernel_spmd(nc, in_maps, core_ids=list(range(8)))

    out = np.zeros((B, S, H), np.float32)
    for cid in range(8):
        b = cid // 4
        out[b] += res.results[cid]["y"]
    return out
